# Initial kernel scaffold
#
"""BiRWKV attention Trainium2 kernel.

Full inputs: r, k, v [4, 4096, 1024] f32; w, u [1, 1, 1024] f32.
Sharding: 8 cores = 4 batches x 2 channel-halves. Each core handles one
batch and 512 channels: [4096, 512] slices.

Per-core algorithm, software-pipelined over 4 c-tiles of 128 channels:
  prep(ct):  DMA k,v,r naturally; PE-transpose 128x128 blocks to PSUM;
             ACT exp -> ek ([c,t]); DVE mul -> ekv  (double-buffered)
  scans(ct): DVE tensor_tensor_scan x4 (fwd/bwd x A/B; bwd via negative
             -stride APs; zero guard columns fold the shift)
  combine(ct): PE float32r identity/diag matmuls accumulate
             num = I@Af + I@Ab + diag(eu)@ekv, den = ... into PSUM;
             DVE reciprocal_approx_fast(den); ACT copies num/wkvT out of
             PSUM; GpSimd muls; PE transposes wkv back to [t,c]; DMA out.

prep(ct+1) is emitted before scans(ct)/combine(ct) so each engine's
in-order stream overlaps c-tiles (PE transposes run during DVE scans).
"""
import functools
from contextlib import ExitStack

import numpy as np

import concourse.bass as bass
import concourse.bacc as bacc
import concourse.tile as tile
from concourse import mybir
from concourse.bass_utils import run_bass_kernel_spmd
from concourse.masks import make_identity

B, T, C = 4, 4096, 1024
NCORES = 8
C_LOC = 512          # channels per core
NCT = C_LOC // 128   # c-tiles per core
NT = T // 128        # 128-col t-chunks per c-tile
NCH = T // 512       # 512-col chunks for the combine stage
F32 = mybir.dt.float32
F32R = mybir.dt.float32r
F16 = mybir.dt.float16
MUL = mybir.AluOpType.mult
ADD = mybir.AluOpType.add
AF = mybir.ActivationFunctionType


def build_tile_kernel(ctx: ExitStack, tc: tile.TileContext,
                      k_d, v_d, r_d, w_d, u_d, out_d):
    nc = tc.nc

    singles = ctx.enter_context(tc.tile_pool(name="singles", bufs=1))
    consts = ctx.enter_context(tc.tile_pool(name="consts", bufs=2))
    nat = ctx.enter_context(tc.tile_pool(name="nat", bufs=1))
    bigp = ctx.enter_context(tc.tile_pool(name="bigp", bufs=1))
    rings = ctx.enter_context(tc.tile_pool(name="rings", bufs=2))
    psT = ctx.enter_context(tc.tile_pool(name="psT", bufs=2, space="PSUM"))
    psACC = ctx.enter_context(tc.tile_pool(name="psACC", bufs=1, space="PSUM"))

    ident = singles.tile([128, 128], F32)
    make_identity(nc, ident)
    ident16 = singles.tile([128, 128], F16)
    nc.gpsimd.memset(ident16, 0.0)
    nc.gpsimd.affine_select(
        out=ident16[:], in_=ident16, compare_op=mybir.AluOpType.not_equal,
        fill=1.0, base=0, pattern=[[-1, 128]], channel_multiplier=1)

    state = {}

    def dma_in(ct):
        c0 = ct * 128
        csl = slice(c0, c0 + 128)
        st = {"csl": csl, "ct": ct}
        # per-c-tile constants
        w_t = consts.tile([128, 1], F32, tag="w")
        u_t = consts.tile([128, 1], F32, tag="u")
        nc.sync.dma_start(w_t, w_d[csl].unsqueeze(1))
        nc.sync.dma_start(u_t, u_d[csl].unsqueeze(1))
        ew = consts.tile([128, 1], F32, tag="ew")
        nc.scalar.activation(ew, w_t, AF.Exp)
        eu = consts.tile([128, 1], F32, tag="eu")
        nc.scalar.activation(eu, u_t, AF.Exp)
        diag_eu = consts.tile([128, 128], F16, tag="diag")
        nc.vector.tensor_scalar_mul(diag_eu[:], ident16, eu)
        st["diag_r"] = diag_eu[:]
        st["ew_b"] = ew[:].broadcast_to((128, T))

        # natural loads
        k_nat = nat.tile([128, NT, 128], F32, tag="knat")
        v_nat = nat.tile([128, NT, 128], F32, tag="vnat")
        r_nat = nat.tile([128, NT, 128], F32, tag="rnat")
        nc.sync.dma_start(k_nat, k_d[:, csl].rearrange("(n p) c -> p n c", p=128))
        nc.sync.dma_start(v_nat, v_d[:, csl].rearrange("(n p) c -> p n c", p=128))
        nc.sync.dma_start(r_nat, r_d[:, csl].rearrange("(n p) c -> p n c", p=128))
        st["r_nat"] = r_nat
        st["k_nat"], st["v_nat"] = k_nat, v_nat
        return st

    def tex(st):
        k_nat, v_nat = st["k_nat"], st["v_nat"]
        # transpose + exp + ekv  (guard columns 0 and T+1 stay zero)
        par = st["ct"] % 2
        ekx = bigp.tile([128, T + 2], F16, tag=f"ek{par}")
        ekvx = bigp.tile([128, T + 2], F16, tag=f"ekv{par}")
        nc.vector.memset(ekx[:, 0:1], 0.0)
        nc.vector.memset(ekx[:, T + 1:T + 2], 0.0)
        nc.vector.memset(ekvx[:, 0:1], 0.0)
        nc.vector.memset(ekvx[:, T + 1:T + 2], 0.0)
        ek = ekx[:, 1:T + 1]
        ekv = ekvx[:, 1:T + 1]
        for q in range(NCH):
            kT = psT.tile([128, 512], F32, tag="kT")
            vT = psT.tile([128, 512], F32, tag="vT")
            for jj in range(4):
                j = q * 4 + jj
                s5 = slice(jj * 128, (jj + 1) * 128)
                nc.tensor.transpose(kT[:, s5], k_nat[:, j, :], ident)
                nc.tensor.transpose(vT[:, s5], v_nat[:, j, :], ident)
            sq = slice(q * 512, (q + 1) * 512)
            nc.scalar.activation(ek[:, sq], kT, AF.Exp)
            nc.vector.tensor_mul(ekv[:, sq], ek[:, sq], vT)
        st.update(ekx=ekx, ekvx=ekvx, ek=ek, ekv=ekv)

    def scans(st):
        ekx, ekvx, ew_b = st["ekx"], st["ekvx"], st["ew_b"]
        par = st["ct"] % 2
        Af = bigp.tile([128, T], F16, tag=f"Af{par}")
        Bf = bigp.tile([128, T], F16, tag=f"Bf{par}")
        Ab = bigp.tile([128, T], F16, tag=f"Ab{par}")
        Bb = bigp.tile([128, T], F16, tag=f"Bb{par}")
        nc.vector.tensor_tensor_scan(Bf[:, 0:T], ew_b,
                                     ekx[:, 0:T], 0.0, MUL, ADD)
        nc.vector.tensor_tensor_scan(Bb[:, T - 1::-1], ew_b,
                                     ekx[:, T + 1:1:-1], 0.0, MUL, ADD)
        nc.vector.tensor_tensor_scan(Af[:, 0:T], ew_b,
                                     ekvx[:, 0:T], 0.0, MUL, ADD)
        nc.vector.tensor_tensor_scan(Ab[:, T - 1::-1], ew_b,
                                     ekvx[:, T + 1:1:-1], 0.0, MUL, ADD)
        st.update(Af=Af, Bf=Bf, Ab=Ab, Bb=Bb)

    def combine(st):
        Af, Bf, Ab, Bb = st["Af"], st["Bf"], st["Ab"], st["Bb"]
        ek, ekv, diag_r = st["ek"], st["ekv"], st["diag_r"]
        r_nat = st["r_nat"]
        sig = r_nat
        nc.scalar.activation(sig.rearrange("p n c -> p (n c)"),
                             r_nat.rearrange("p n c -> p (n c)"), AF.Sigmoid)
        out_nat = nat.tile([128, NT, 128], F32, tag="onat")
        for q in range(NCH):
            sq = slice(q * 512, (q + 1) * 512)
            num = psACC.tile([128, 512], F32, tag="num")
            den = psACC.tile([128, 512], F32, tag="den")
            for dst, s2, s3 in ((den, Bf, Bb), (num, Af, Ab)):
                nc.tensor.matmul(dst, ident16[:], s2[:, sq],
                                 start=True, stop=False)
                nc.tensor.matmul(dst, ident16[:], s3[:, sq],
                                 start=False, stop=False)
            nc.tensor.matmul(den, diag_r, ek[:, sq],
                             start=False, stop=True)
            nc.tensor.matmul(num, diag_r, ekv[:, sq],
                             start=False, stop=True)
            recip = rings.tile([128, 512], F32, tag="recip")
            nc.vector.reciprocal_approx_fast(out=recip, in_=den[:])
            num_sb = rings.tile([128, 512], F32, tag="numsb", bufs=1)
            nc.scalar.copy(num_sb, num)
            wkv = rings.tile([128, 512], F16, tag="wkv", bufs=1)
            nc.gpsimd.tensor_tensor(wkv, num_sb, recip, op=MUL)
            wkvT = psT.tile([128, 512], F16, tag="wkvT")
            for jj in range(4):
                s5 = slice(jj * 128, (jj + 1) * 128)
                nc.tensor.transpose(wkvT[:, s5], wkv[:, s5], ident16[:])
            wkvT_sb = rings.tile([128, 512], F16, tag="wkvTsb")
            nc.scalar.copy(wkvT_sb, wkvT)
            nc.gpsimd.tensor_tensor(
                out_nat[:, q * 4:(q + 1) * 4, :].rearrange("p n c -> p (n c)"),
                sig[:, q * 4:(q + 1) * 4, :].rearrange("p n c -> p (n c)"),
                wkvT_sb, op=MUL)
        nc.sync.dma_start(
            out_d[:, st["csl"]].rearrange("(n p) c -> p n c", p=128), out_nat)

    # software pipeline: dma(ct+1) early; transposes/exp/ekv(ct+1)
    # emitted between scans(ct) and combine(ct) so PE runs them during
    # the next scans while MMs(ct) follow immediately after.
    state[0] = dma_in(0)
    tex(state[0])
    for ct in range(NCT):
        if ct + 1 < NCT:
            state[ct + 1] = dma_in(ct + 1)
        scans(state[ct])
        if ct + 1 < NCT:
            tex(state[ct + 1])
        combine(state[ct])
        del state[ct]


@functools.lru_cache(maxsize=1)
def get_nc():
    nc = bacc.Bacc("TRN2", target_bir_lowering=False, debug=False,
                   enable_asserts=False, num_devices=NCORES)
    k_d = nc.dram_tensor("k", [T, C_LOC], F32, kind="ExternalInput").ap()
    v_d = nc.dram_tensor("v", [T, C_LOC], F32, kind="ExternalInput").ap()
    r_d = nc.dram_tensor("r", [T, C_LOC], F32, kind="ExternalInput").ap()
    w_d = nc.dram_tensor("w", [C_LOC], F32, kind="ExternalInput").ap()
    u_d = nc.dram_tensor("u", [C_LOC], F32, kind="ExternalInput").ap()
    out_d = nc.dram_tensor("out", [T, C_LOC], F32, kind="ExternalOutput").ap()
    with tile.TileContext(nc) as tc:
        with ExitStack() as ctx:
            build_tile_kernel(ctx, tc, k_d, v_d, r_d, w_d, u_d, out_d)
    nc.compile()
    return nc


def _in_maps(r, k, v, w, u):
    maps = []
    for core in range(NCORES):
        b, h = divmod(core, 2)
        cs = slice(h * C_LOC, (h + 1) * C_LOC)
        maps.append({
            "k": np.ascontiguousarray(k[b, :, cs]),
            "v": np.ascontiguousarray(v[b, :, cs]),
            "r": np.ascontiguousarray(r[b, :, cs]),
            "w": np.ascontiguousarray(w.reshape(-1)[cs]),
            "u": np.ascontiguousarray(u.reshape(-1)[cs]),
        })
    return maps


def kernel(r, k, v, w, u, trace=False):
    nc = get_nc()
    res = run_bass_kernel_spmd(nc, _in_maps(r, k, v, w, u),
                               core_ids=list(range(NCORES)), trace=trace)
    out = np.empty((B, T, C), np.float32)
    for core in range(NCORES):
        b, h = divmod(core, 2)
        out[b, :, h * C_LOC:(h + 1) * C_LOC] = res.results[core]["out"]
    if trace:
        kernel.last_results = res
    return out



# revision 26
# speedup vs baseline: 1.3535x; 1.3535x over previous
"""BiRWKV attention Trainium2 kernel (v8 = v2 structure + validated wins).

Full inputs: r, k, v [4, 4096, 1024] f32; w, u [1, 1, 1024] f32.
Sharding: 8 cores = 4 batches x 2 channel-halves. Each core handles one
batch and 512 channels: [4096, 512] slices.

Per-core pipeline over 4 c-tiles of 128 channels:
  pre(ct):   Scalar exp(k)->ek16 per 512-chunk (telescopes behind the
             chunked k DMA); v loaded straight to f16 by a GpSimd
             casting DMA; DVE ekv16 = ek16*v16; Scalar sigmoid last.
  T(ct):     PE-transposes ek16/ekv16 (f16, 1-pass) into a f16 PSUM
             tile per 512-chunk; Scalar copies to ekx/ekvx [c,t].
  scans(ct): DVE tensor_tensor_scan x4 with ew materialized stride-1.
  comb(ct):  per 512-chunk: f16 identity/diag matmuls -> num/den (f32
             PSUM); DVE recip; DVE wkv16 = num(PSUM)*recip; PE
             transposes wkv back; DVE out16 = sig16*wkvT(PSUM) (f16,
             2x mode); GpSimd casting DMA upcasts to f32 on the way out.
T(ct+1) is interleaved into comb(ct) so PE streams transposes while DVE
drains the combine; scans see inputs staged entirely by PE+Scalar.
"""
import functools
from contextlib import ExitStack

import numpy as np

import concourse.bass as bass
import concourse.bacc as bacc
import concourse.tile as tile
from concourse import mybir
from concourse.bass_utils import run_bass_kernel_spmd

B, T, C = 4, 4096, 1024
NCORES = 8
C_LOC = 512          # channels per core
NCT = C_LOC // 128   # c-tiles per core
NT = T // 128        # 128-col t-chunks per c-tile
NCH = T // 512       # 512-col chunks
F32 = mybir.dt.float32
F16 = mybir.dt.float16
MUL = mybir.AluOpType.mult
ADD = mybir.AluOpType.add
AF = mybir.ActivationFunctionType


def build_tile_kernel(ctx: ExitStack, tc: tile.TileContext,
                      k_d, v_d, r_d, w_d, u_d, out_d):
    nc = tc.nc

    singles = ctx.enter_context(tc.tile_pool(name="singles", bufs=1))
    nat = ctx.enter_context(tc.tile_pool(name="nat", bufs=1))
    nat16 = ctx.enter_context(tc.tile_pool(name="nat16", bufs=1))
    sigp = ctx.enter_context(tc.tile_pool(name="sigp", bufs=2))
    bigp = ctx.enter_context(tc.tile_pool(name="bigp", bufs=1))
    psT = ctx.enter_context(tc.tile_pool(name="psT", bufs=2, space="PSUM"))
    psW = ctx.enter_context(tc.tile_pool(name="psW", bufs=2, space="PSUM"))
    psACC = ctx.enter_context(tc.tile_pool(name="psACC", bufs=2, space="PSUM"))

    ident16 = singles.tile([128, 128], F16)
    nc.gpsimd.memset(ident16, 0.0)
    nc.gpsimd.affine_select(
        out=ident16[:], in_=ident16, compare_op=mybir.AluOpType.not_equal,
        fill=1.0, base=0, pattern=[[-1, 128]], channel_multiplier=1)

    # per-c-tile constants hoisted to startup
    w_all = singles.tile([128, NCT], F32)
    u_all = singles.tile([128, NCT], F32)
    nc.sync.dma_start(w_all, w_d.rearrange("(ct p) -> p ct", p=128))
    nc.sync.dma_start(u_all, u_d.rearrange("(ct p) -> p ct", p=128))
    ew_all = singles.tile([128, NCT], F32)
    eu_all = singles.tile([128, NCT], F32)
    nc.scalar.activation(ew_all, w_all, AF.Exp)
    nc.scalar.activation(eu_all, u_all, AF.Exp)
    diags = []
    for ct in range(NCT):
        diag_eu = singles.tile([128, 128], F16, tag=f"diag{ct}")
        nc.vector.tensor_scalar_mul(diag_eu[:], ident16,
                                    eu_all[:, ct:ct + 1])
        diags.append(diag_eu)

    state = {}

    def dma_in(ct):
        c0 = ct * 128
        csl = slice(c0, c0 + 128)
        st = {"csl": csl, "ct": ct,
              "diag_r": diags[ct][:], "ew": ew_all[:, ct:ct + 1]}
        v16 = nat16.tile([128, T], F16, tag="v16")
        vview = v_d[:, csl].rearrange("(n p) c -> p n c", p=128)
        v16v = v16.rearrange("p (n c) -> p n c", c=128)
        nc.gpsimd.dma_start(v16v[:, 0:NT // 2, :], vview[:, 0:NT // 2, :])
        nc.gpsimd.dma_start(v16v[:, NT // 2:, :], vview[:, NT // 2:, :])
        k_nat = nat.tile([128, NT, 128], F32, tag="knat")
        kview = k_d[:, csl].rearrange("(n p) c -> p n c", p=128)
        for q in range(NCH):
            nsl = slice(q * 4, (q + 1) * 4)
            nc.sync.dma_start(k_nat[:, nsl, :], kview[:, nsl, :])
        r_nat = nat.tile([128, NT, 128], F32, tag="rnat")
        nc.sync.dma_start(r_nat, r_d[:, csl].rearrange("(n p) c -> p n c", p=128))
        st["k_nat"], st["v16"], st["r_nat"] = k_nat, v16, r_nat
        return st

    def pre(st):
        """Chunked Scalar exp + chunked DVE ekv so staging telescopes
        behind the chunked k DMA. ew_full/sigmoid are emitted later
        (pre_tail) so they don't delay the staging copies on Scalar."""
        k_nat = st["k_nat"]
        ek16 = nat16.tile([128, T], F16, tag="ek16")
        ekv16 = nat16.tile([128, T], F16, tag="ekv16")
        kf = k_nat.rearrange("p n c -> p (n c)")
        for q in range(NCH):
            sq = slice(q * 512, (q + 1) * 512)
            nc.scalar.activation(ek16[:, sq], kf[:, sq], AF.Exp)
            nc.gpsimd.tensor_tensor(ekv16[:, sq], ek16[:, sq],
                                    st["v16"][:, sq], op=MUL)
        st["ek16"], st["ekv16"] = ek16, ekv16

        par = st["ct"] % 2
        ekx = bigp.tile([128, T + 2], F16, tag=f"ek{par}")
        ekvx = bigp.tile([128, T + 2], F16, tag=f"ekv{par}")
        nc.vector.memset(ekx[:, 0:1], 0.0)
        nc.vector.memset(ekx[:, T + 1:T + 2], 0.0)
        nc.vector.memset(ekvx[:, 0:1], 0.0)
        nc.vector.memset(ekvx[:, T + 1:T + 2], 0.0)
        st.update(ekx=ekx, ekvx=ekvx, ek=ekx[:, 1:T + 1], ekv=ekvx[:, 1:T + 1])

    def pre_tail(st):
        ew_full = bigp.tile([128, T], F16, tag="ewfull")
        nc.scalar.activation(ew_full[:], st["ek16"][:], AF.Identity,
                             scale=0.0, bias=st["ew"])
        st["ew_full"] = ew_full
        sig16 = sigp.tile([128, T], F16, tag="sig16")
        nc.scalar.activation(sig16[:],
                             st["r_nat"].rearrange("p n c -> p (n c)"),
                             AF.Sigmoid)
        st["sig16"] = sig16

    def trans_q(st, q):
        ek16, ekv16 = st["ek16"], st["ekv16"]
        bigT = psT.tile([128, 1024], F16, tag="bigT")
        for jj in range(4):
            j = q * 4 + jj
            s5 = slice(jj * 128, (jj + 1) * 128)
            s6 = slice(512 + jj * 128, 512 + (jj + 1) * 128)
            nc.tensor.transpose(bigT[:, s5], ek16[:, j * 128:(j + 1) * 128],
                                ident16)
            nc.tensor.transpose(bigT[:, s6], ekv16[:, j * 128:(j + 1) * 128],
                                ident16)
        sq = slice(q * 512, (q + 1) * 512)
        nc.scalar.copy(st["ek"][:, sq], bigT[:, 0:512])
        nc.scalar.copy(st["ekv"][:, sq], bigT[:, 512:1024])

    def scans(st):
        ekx, ekvx = st["ekx"], st["ekvx"]
        ew_b = st["ew_full"][:]
        Af = bigp.tile([128, T], F16, tag="Af")
        Bf = bigp.tile([128, T], F16, tag="Bf")
        Ab = bigp.tile([128, T], F16, tag="Ab")
        Bb = bigp.tile([128, T], F16, tag="Bb")
        nc.vector.tensor_tensor_scan(Bf[:, 0:T], ew_b,
                                     ekx[:, 0:T], 0.0, MUL, ADD)
        nc.vector.tensor_tensor_scan(Bb[:, T - 1::-1], ew_b,
                                     ekx[:, T + 1:1:-1], 0.0, MUL, ADD)
        nc.vector.tensor_tensor_scan(Af[:, 0:T], ew_b,
                                     ekvx[:, 0:T], 0.0, MUL, ADD)
        nc.vector.tensor_tensor_scan(Ab[:, T - 1::-1], ew_b,
                                     ekvx[:, T + 1:1:-1], 0.0, MUL, ADD)
        st.update(Af=Af, Bf=Bf, Ab=Ab, Bb=Bb)

    def comb_q(st, q, out16):
        Af, Bf, Ab, Bb = st["Af"], st["Bf"], st["Ab"], st["Bb"]
        ek, ekv, diag_r = st["ek"], st["ekv"], st["diag_r"]
        sq = slice(q * 512, (q + 1) * 512)
        num = psACC.tile([128, 512], F32, tag="num")
        den = psACC.tile([128, 512], F32, tag="den")
        for dst, s2, s3 in ((den, Bf, Bb), (num, Af, Ab)):
            nc.tensor.matmul(dst, ident16[:], s2[:, sq],
                             start=True, stop=False)
            nc.tensor.matmul(dst, ident16[:], s3[:, sq],
                             start=False, stop=False)
        nc.tensor.matmul(den, diag_r, ek[:, sq], start=False, stop=True)
        nc.tensor.matmul(num, diag_r, ekv[:, sq], start=False, stop=True)
        recip = sigp.tile([128, 512], F32, tag="recip")
        nc.vector.reciprocal_approx_fast(out=recip, in_=den[:])
        wkv = sigp.tile([128, 512], F16, tag="wkv")
        nc.vector.tensor_mul(wkv[:], num[:], recip[:])
        wkvT = psW.tile([128, 512], F16, tag="wkvT")
        for jj in range(4):
            s5 = slice(jj * 128, (jj + 1) * 128)
            nc.tensor.transpose(wkvT[:, s5], wkv[:, s5], ident16[:])
        nc.vector.tensor_mul(
            out16[:, q * 4:(q + 1) * 4, :].rearrange("p n c -> p (n c)"),
            st["sig16"][:, q * 512:(q + 1) * 512],
            wkvT[:])

    def comb(st, nxt):
        out16 = nat16.tile([128, NT, 128], F16, tag="out16")
        for q in range(NCH):
            if nxt is not None:
                trans_q(nxt, q)
            comb_q(st, q, out16)
        nc.gpsimd.dma_start(
            out_d[:, st["csl"]].rearrange("(n p) c -> p n c", p=128), out16)

    # software pipeline
    state[0] = dma_in(0)
    pre(state[0])
    for q in range(NCH):
        trans_q(state[0], q)
    pre_tail(state[0])
    for ct in range(NCT):
        if ct + 1 < NCT:
            state[ct + 1] = dma_in(ct + 1)
        scans(state[ct])
        if ct + 1 < NCT:
            pre(state[ct + 1])
            comb(state[ct], state[ct + 1])
            pre_tail(state[ct + 1])
        else:
            comb(state[ct], None)
        del state[ct]


@functools.lru_cache(maxsize=1)
def get_nc():
    nc = bacc.Bacc("TRN2", target_bir_lowering=False, debug=False,
                   enable_asserts=False, num_devices=NCORES)
    k_d = nc.dram_tensor("k", [T, C_LOC], F32, kind="ExternalInput").ap()
    v_d = nc.dram_tensor("v", [T, C_LOC], F32, kind="ExternalInput").ap()
    r_d = nc.dram_tensor("r", [T, C_LOC], F32, kind="ExternalInput").ap()
    w_d = nc.dram_tensor("w", [C_LOC], F32, kind="ExternalInput").ap()
    u_d = nc.dram_tensor("u", [C_LOC], F32, kind="ExternalInput").ap()
    out_d = nc.dram_tensor("out", [T, C_LOC], F32, kind="ExternalOutput").ap()
    with tile.TileContext(nc) as tc:
        with ExitStack() as ctx:
            build_tile_kernel(ctx, tc, k_d, v_d, r_d, w_d, u_d, out_d)
    nc.compile()
    return nc


def _in_maps(r, k, v, w, u):
    maps = []
    for core in range(NCORES):
        b, h = divmod(core, 2)
        cs = slice(h * C_LOC, (h + 1) * C_LOC)
        maps.append({
            "k": np.ascontiguousarray(k[b, :, cs]),
            "v": np.ascontiguousarray(v[b, :, cs]),
            "r": np.ascontiguousarray(r[b, :, cs]),
            "w": np.ascontiguousarray(w.reshape(-1)[cs]),
            "u": np.ascontiguousarray(u.reshape(-1)[cs]),
        })
    return maps


def kernel(r, k, v, w, u, trace=False):
    nc = get_nc()
    res = run_bass_kernel_spmd(nc, _in_maps(r, k, v, w, u),
                               core_ids=list(range(NCORES)), trace=trace)
    out = np.empty((B, T, C), np.float32)
    for core in range(NCORES):
        b, h = divmod(core, 2)
        out[b, :, h * C_LOC:(h + 1) * C_LOC] = res.results[core]["out"]
    if trace:
        kernel.last_results = res
    return out
